# revision 1
# baseline (speedup 1.0000x reference)
"""Multi-head self-attention Trainium2 kernel, sharded over 8 NeuronCores.

Sharding: core = (batch, head_group): 2 batches x 4 head-groups (4 heads each).
Each core computes qkv for its batch restricted to its heads (tensor-parallel
column slice), full-sequence attention for those heads, and a row-parallel
slice of the output projection, producing a partial [T, C] output.
Host: out[b] = sum of the 4 head-group partials + b_proj.
"""

import math
import sys

import numpy as np

sys.path.insert(0, "/opt/trn_rl_repo")

import concourse.bacc as bacc
import concourse.bass as bass
import concourse.tile as tile
from concourse import mybir
from concourse.bass_utils import run_bass_kernel_spmd

B, T, C = 2, 2048, 1024
NH, DH = 16, 64
HG = 4                 # heads per core
DL = HG * DH           # 256 local qk channels
DV = HG * (DH + 1)     # 260: v columns + one ones-column per head
N_CORES = 8

F32 = mybir.dt.float32
F32R = mybir.dt.float32r
F16 = mybir.dt.float16

SCALE = 1.0 / math.sqrt(DH)


def build_bass():
    nc = bacc.Bacc("TRN2", target_bir_lowering=False, debug=False)

    x_in = nc.declare_dram_parameter("x_loc", [T, C], F32, isOutput=False)
    w_qk = nc.declare_dram_parameter("w_qk", [C, 2 * DL], F32R, isOutput=False)
    w_v = nc.declare_dram_parameter("w_v_ext", [C, DV], F32R, isOutput=False)
    b_qk = nc.declare_dram_parameter("b_qk", [128, 4], F32, isOutput=False)
    b_v = nc.declare_dram_parameter("b_v_ext", [1, DV], F32R, isOutput=False)
    w_p = nc.declare_dram_parameter("w_proj_loc", [DL, C], F16, isOutput=False)
    iden = nc.declare_dram_parameter("identity", [128, 128], F32, isOutput=False)
    ones = nc.declare_dram_parameter("ones_row", [1, T], F32R, isOutput=False)
    out = nc.declare_dram_parameter("out_partial", [T, C], F32, isOutput=True)

    Exp = mybir.ActivationFunctionType.Exp

    with tile.TileContext(nc) as tc:
        with (
            tc.tile_pool(name="singles", bufs=1) as singles,
            tc.tile_pool(name="xload", bufs=5) as xload,
            tc.tile_pool(name="pt", bufs=5) as ptp,
            tc.tile_pool(name="osmall", bufs=3) as osmall,
            tc.tile_pool(name="psmm", bufs=2, space="PSUM") as psmm,
            # av shares psmm
            tc.tile_pool(name="pssc", bufs=3, space="PSUM") as pssc,
        ):
            iden_sb = singles.tile([128, 128], F32)
            nc.sync.dma_start(out=iden_sb[:], in_=iden[:])
            ones_sb = singles.tile([1, T], F32R)
            nc.sync.dma_start(out=ones_sb[:], in_=ones[:])
            bqk_sb = singles.tile([128, 4], F32)
            nc.sync.dma_start(out=bqk_sb[:], in_=b_qk[:])
            bv_sb = singles.tile([1, DV], F32R)
            nc.sync.dma_start(out=bv_sb[:], in_=b_v[:])
            # broadcast V bias row to all 128 partitions (done once)
            bvb_ps = psmm.tile([128, DV], F32, tag="mm", name="bvb_ps")
            nc.tensor.matmul(
                bvb_ps[:], lhsT=ones_sb[:, :128], rhs=bv_sb[:],
                start=True, stop=True,
            )
            bvb = singles.tile([128, DV], F32)
            nc.vector.tensor_copy(bvb[:], bvb_ps[:])

            xa0 = []
            for k in range(4):
                a0 = xload.tile([128, C], F32, tag="xa", bufs=4, name=f"xa0_{k}")
                nc.sync.dma_start(out=a0[:], in_=x_in[k * 128:(k + 1) * 128, :])
                xa0.append(a0)

            wqk_sb = []
            wv_sb = []
            for ci in range(8):
                t_qk = singles.tile([128, 2 * DL], F32R, name=f"wqk{ci}")
                nc.sync.dma_start(out=t_qk[:], in_=w_qk[ci * 128:(ci + 1) * 128, :])
                wqk_sb.append(t_qk)
                t_v = singles.tile([128, DV], F32R, name=f"wv{ci}")
                nc.sync.dma_start(out=t_v[:], in_=w_v[ci * 128:(ci + 1) * 128, :])
                wv_sb.append(t_v)
            wp_sb = []
            for di in range(2):
                t_p = singles.tile([128, C], F16, name=f"wp{di}")
                nc.sync.dma_start(out=t_p[:], in_=w_p[di * 128:(di + 1) * 128, :])
                wp_sb.append(t_p)

            # ---- Phases A+B streamed per t-block of 512 ------------------
            xt = [singles.tile([128, T], F32R, name=f"xt{ci}") for ci in range(8)]
            qkt = [singles.tile([128, T], F32R, name=f"qkt{m}") for m in range(4)]
            v_sb = [singles.tile([128, DV], F16, name=f"v{tt}") for tt in range(16)]

            def qk_block(m, tb):
                ps = pssc.tile([128, 512], F32, tag="sc", name=f"qkps{m}_{tb}")
                for ci in range(8):
                    nc.tensor.matmul(
                        ps[:],
                        lhsT=wqk_sb[ci][:, m * 128:(m + 1) * 128],
                        rhs=xt[ci][:, tb * 512:(tb + 1) * 512],
                        start=(ci == 0),
                        stop=(ci == 7),
                    )
                dst = qkt[m][:, tb * 512:(tb + 1) * 512]
                nc.vector.tensor_scalar_add(dst, ps[:], bqk_sb[:, m:m + 1])

            def tb_group(tb):
                # load + transpose x for this t block
                if tb == 0:
                    xa = xa0
                else:
                    xa = []
                    for k in range(4):
                        tt = tb * 4 + k
                        a = xload.tile([128, C], F32, tag="xa", bufs=4,
                                       name=f"xa{tb}_{k}")
                        nc.sync.dma_start(out=a[:], in_=x_in[tt * 128:(tt + 1) * 128, :])
                        xa.append(a)
                for ci in range(8):
                    ps = pssc.tile([128, 512], F32, tag="sc", name=f"tp{tb}_{ci}")
                    for k in range(4):
                        nc.tensor.transpose(
                            ps[:, k * 128:(k + 1) * 128],
                            xa[k][:, ci * 128:(ci + 1) * 128],
                            iden_sb[:],
                        )
                    nc.vector.tensor_copy(
                        xt[ci][:, tb * 512:(tb + 1) * 512], ps[:])
                # K projections for this t block (attention consumes K in st order)
                qk_block(2, tb)
                qk_block(3, tb)
                # V for this t block
                for tt in range(tb * 4, tb * 4 + 4):
                    ps = pssc.tile([128, DV], F32, tag="sc", name=f"vps{tt}")
                    for ci in range(8):
                        nc.tensor.matmul(
                            ps[:],
                            lhsT=xt[ci][:, tt * 128:(tt + 1) * 128],
                            rhs=wv_sb[ci][:],
                            start=(ci == 0),
                            stop=(ci == 7),
                        )
                    nc.vector.tensor_add(v_sb[tt][:], ps[:], bvb[:])

            # ---- Phases C+D interleaved over q-blocks ----------------------
            # O_T fp16 [2*DL, T]: 2 tiles of [128, T]
            ot = [singles.tile([128, T], F16, name=f"ot{di}") for di in range(2)]
            def proj_tt(tt):
                o_out = xload.tile([128, C], F32, tag="oout", name=f"oout{tt}", bufs=3)
                for nb in range(2):
                    ps = psmm.tile([128, 512], F32, tag="mm", name=f"prps{tt}_{nb}")
                    for di in range(2):
                        nc.tensor.matmul(
                            ps[:],
                            lhsT=ot[di][:, tt * 128:(tt + 1) * 128],
                            rhs=wp_sb[di][:, nb * 512:(nb + 1) * 512],
                            start=(di == 0),
                            stop=(di == 1),
                        )
                    nc.vector.tensor_copy(o_out[:, nb * 512:(nb + 1) * 512], ps[:])
                nc.sync.dma_start(
                    out=out[tt * 128:(tt + 1) * 128, :],
                    in_=o_out[:],
                )

            pending_epi = [None]

            def emit_epilogue():
                if pending_epi[0] is None:
                    return
                h, qb, av = pending_epi[0]
                pending_epi[0] = None
                # divide by softmax sums (row 64), write O_T
                sums_sb = osmall.tile([1, 512], F32R, tag="sums")
                nc.vector.tensor_copy(sums_sb[:], av[DH:DH + 1, :])
                bc = pssc.tile([64, 512], F32, tag="sc", name=f"bc{h}_{qb}")
                nc.tensor.matmul(
                    bc[:],
                    lhsT=ones_sb[:, :64],
                    rhs=sums_sb[:],
                    start=True,
                    stop=True,
                )
                rec = osmall.tile([64, 512], F32, tag="rec")
                nc.vector.reciprocal(rec[:], bc[:])
                nc.vector.tensor_mul(
                    ot[h // 2][moff_of(h):moff_of(h) + 64,
                               qb * 512:(qb + 1) * 512],
                    av[0:DH, :],
                    rec[:],
                )

            def moff_of(h):
                return (h % 2) * 64

            LOOK = 2   # pairs of lookahead between scores/exp and AV

            class Unit:
                def __init__(self, h, qb):
                    self.h, self.qb = h, qb
                    self.q_tile = qkt[h // 2]
                    self.k_tile = qkt[2 + h // 2]
                    self.moff = moff_of(h)
                    self.av = None
                    self.pts = []
                    self.sc_done = 0
                    self.av_done = 0

                def _emit_scores_pair(self):
                    p = self.sc_done
                    h, qb = self.h, self.qb
                    ps = pssc.tile([128, 1024], F32, tag="sc",
                                   name=f"sc{h}_{qb}_{p}")
                    for half in range(2):
                        st = 2 * p + half
                        nc.tensor.matmul(
                            ps[:, half * 512:(half + 1) * 512],
                            lhsT=self.k_tile[self.moff:self.moff + 64,
                                             st * 128:(st + 1) * 128],
                            rhs=self.q_tile[self.moff:self.moff + 64,
                                            qb * 512:(qb + 1) * 512],
                            start=True,
                            stop=True,
                        )
                    pt = ptp.tile([128, 1024], F16, tag="pt", name=f"pt{h}_{qb}_{p}")
                    nc.scalar.activation(pt[:], ps[:], Exp, scale=SCALE)
                    self.pts.append(pt)
                    self.sc_done += 1

                def _emit_av_pair(self):
                    sp = self.av_done
                    h, qb = self.h, self.qb
                    if self.av is None:
                        self.av = psmm.tile([DH + 1, 512], F32, tag="mm",
                                            name=f"av{h}_{qb}")
                    ptk = self.pts[sp]
                    for half in range(2):
                        st = 2 * sp + half
                        nc.tensor.matmul(
                            self.av[:],
                            lhsT=v_sb[st][:, h * (DH + 1):(h + 1) * (DH + 1)],
                            rhs=ptk[:, half * 512:(half + 1) * 512],
                            start=(st == 0),
                            stop=(st == 15),
                        )
                    self.av_done += 1

                def emit(self, n_pairs):
                    for _ in range(n_pairs):
                        if self.sc_done < 8:
                            self._emit_scores_pair()
                        if self.sc_done == 2 and self.av_done == 0:
                            emit_epilogue()
                        if self.sc_done - self.av_done > LOOK or \
                           (self.sc_done == 8 and self.av_done < 8 and
                            self.sc_done - self.av_done > LOOK):
                            self._emit_av_pair()

                def finish(self):
                    while self.sc_done < 8 or self.av_done < 8:
                        if self.sc_done < 8:
                            self._emit_scores_pair()
                            if self.sc_done == 2 and self.av_done == 0:
                                emit_epilogue()
                        else:
                            self._emit_av_pair()
                    emit_epilogue()
                    pending_epi[0] = (self.h, self.qb, self.av)

            def attn_unit(h, qb):
                u = Unit(h, qb)
                u.finish()

            tb_group(0)
            qk_block(0, 0)
            qk_block(1, 0)
            u00 = Unit(0, 0)
            u10 = Unit(1, 0)
            u00.emit(2)
            tb_group(1)
            u00.emit(2)
            u10.emit(2)
            tb_group(2)
            u00.emit(2)
            u10.emit(2)
            tb_group(3)
            u00.emit(2)
            u10.emit(2)
            u00.finish()
            u10.finish()
            for qb in range(4):          # q blocks of 512
                for h in range(HG):
                    if not (qb == 0 and h < 2):
                        attn_unit(h, qb)
                    # PE filler while ACT runs exp:
                    if h == 0 and qb < 3:
                        qk_block(0, qb + 1)
                    if h == 1 and qb < 3:
                        qk_block(1, qb + 1)
                    if h >= 2 and qb > 0:
                        base = (qb - 1) * 4 + (h - 2) * 2
                        proj_tt(base)
                        proj_tt(base + 1)
            emit_epilogue()
            for tt in range(12, 16):
                proj_tt(tt)

    nc.compile()
    return nc


_CACHE = {}


def _get_nc():
    if "nc" not in _CACHE:
        _CACHE["nc"] = build_bass()
    return _CACHE["nc"]


def make_in_maps(x, w_qkv, b_qkv, w_proj):
    identity = np.eye(128, dtype=np.float32)
    ones_row = np.ones((1, T), dtype=np.float32)
    in_maps = []
    for core in range(N_CORES):
        b = core // 4
        hg = core % 4
        cs = slice(hg * DL, (hg + 1) * DL)
        wq = w_qkv[:, 0 * C:1 * C][:, cs]
        wk = w_qkv[:, 1 * C:2 * C][:, cs]
        wv = w_qkv[:, 2 * C:3 * C][:, cs]
        bq = b_qkv[0 * C:1 * C][cs]
        bk = b_qkv[1 * C:2 * C][cs]
        bv = b_qkv[2 * C:3 * C][cs]
        # v extended: per head 64 v-cols + a ones column (softmax denominator)
        w_v_ext = np.zeros((C, DV), dtype=np.float32)
        b_v_ext = np.zeros((1, DV), dtype=np.float32)
        for hh in range(HG):
            w_v_ext[:, hh * (DH + 1):hh * (DH + 1) + DH] = wv[:, hh * DH:(hh + 1) * DH]
            b_v_ext[0, hh * (DH + 1):hh * (DH + 1) + DH] = bv[hh * DH:(hh + 1) * DH]
            b_v_ext[0, hh * (DH + 1) + DH] = 1.0
        in_maps.append({
            "x_loc": np.ascontiguousarray(x[b]),
            "w_qk": np.ascontiguousarray(np.concatenate([wq, wk], axis=1)),
            "w_v_ext": w_v_ext,
            "b_qk": np.stack(
                [np.concatenate([bq, bk])[m * 128:(m + 1) * 128] for m in range(4)],
                axis=1).astype(np.float32),
            "b_v_ext": b_v_ext,
            "w_proj_loc": np.ascontiguousarray(w_proj[cs, :]).astype(np.float16),
            "identity": identity,
            "ones_row": ones_row,
        })
    return in_maps


def kernel(x, w_qkv, b_qkv, w_proj, b_proj, **runner_kwargs):
    x = np.asarray(x, dtype=np.float32)
    w_qkv = np.asarray(w_qkv, dtype=np.float32)
    b_qkv = np.asarray(b_qkv, dtype=np.float32)
    w_proj = np.asarray(w_proj, dtype=np.float32)
    b_proj = np.asarray(b_proj, dtype=np.float32)

    nc = _get_nc()
    in_maps = make_in_maps(x, w_qkv, b_qkv, w_proj)
    res = run_bass_kernel_spmd(nc, in_maps, list(range(N_CORES)), **runner_kwargs)
    parts = [res.results[i]["out_partial"] for i in range(N_CORES)]
    outv = np.zeros((B, T, C), dtype=np.float32)
    for b in range(B):
        outv[b] = parts[4 * b + 0] + parts[4 * b + 1] + parts[4 * b + 2] + parts[4 * b + 3]
        outv[b] += b_proj[None, :]
    if runner_kwargs:
        return outv, res
    return outv


if __name__ == "__main__":
    import reference

    inputs = reference.setup_inputs()
    inputs = {k: np.asarray(v) for k, v in inputs.items()}
    got = kernel(**inputs)
    want = np.asarray(reference.reference(**inputs))
    err = np.abs(got - want).max() / np.abs(want).max()
    print("rel err:", err)



# revision 9
# speedup vs baseline: 1.2671x; 1.2671x over previous
"""Multi-head self-attention Trainium2 kernel, sharded over 8 NeuronCores.

Sharding: core = (batch, head_group): 2 batches x 4 head-groups (4 heads each).
Each core computes qkv for its batch restricted to its heads, full-sequence
attention for those heads, and a row-parallel slice of the output projection,
producing a partial [T, C] output (fp16). Host: out[b] = sum of the 4
head-group partials + b_eff where b_eff folds b_proj and the V bias.

v2 design notes (all relative to the fp32/on-chip-transpose baseline):
  - x is transposed, packed and cast to fp16 on the host; no on-chip
    transposes or x^T copies are needed.
  - K bias is dropped entirely (softmax is invariant to per-query constants,
    and q.bk is per-query); V bias is folded into b_proj on the host
    (sum_s w_s = 1); only the Q bias is applied on-chip.
  - AV is computed transposed: out[q, d] = sum_s P[s,q] V[s,d] with
    ap_size=65 per chunk matmul, which halves the PE cost of AV and makes
    the softmax divide a single per-partition tensor_scalar divide.
  - The softmax denominator comes from a ones-column appended per head in
    the V tile (memset once).
  - Everything on the PE runs fp16 (1.0 cycles/row); fp8 was measured to
    break the 2e-2 gate (diffuse attention preserves per-key noise).
"""

import math
import sys

import numpy as np

sys.path.insert(0, "/opt/trn_rl_repo")

import concourse.bacc as bacc
import concourse.bass as bass
import concourse.tile as tile
from concourse import mybir
from concourse.bass_utils import run_bass_kernel_spmd

B, T, C = 2, 2048, 1024
NH, DH = 16, 64
HG = 4                  # heads per core
DL = HG * DH            # 256 local head dims
N_CORES = 8

F32 = mybir.dt.float32
F16 = mybir.dt.float16

SCALE = 1.0 / math.sqrt(DH)
Exp = mybir.ActivationFunctionType.Exp


def build_bass():
    nc = bacc.Bacc("TRN2", target_bir_lowering=False, debug=False)

    # host-packed params: [p, ci*w + j] = w[ci*128 + p, j]
    x_in = nc.declare_dram_parameter("x_pack", [128, 8 * T], F16, isOutput=False)
    wk_in = nc.declare_dram_parameter("wk_pack", [128, 8 * DL], F16, isOutput=False)
    wq_in = nc.declare_dram_parameter("wq_pack", [128, 8 * DL], F16, isOutput=False)
    wv_in = nc.declare_dram_parameter("wv_pack", [128, 8 * DL], F16, isOutput=False)
    wp_in = nc.declare_dram_parameter("wp_pack", [128, 2 * C], F16, isOutput=False)
    bq_in = nc.declare_dram_parameter("b_q", [128, 2], F32, isOutput=False)
    id_in = nc.declare_dram_parameter("iden16", [128, 128], F16, isOutput=False)
    out = nc.declare_dram_parameter("out_partial", [T, C], F16, isOutput=True)

    with tile.TileContext(nc) as tc:
        with (
            tc.tile_pool(name="singles", bufs=1) as singles,
            tc.tile_pool(name="pt", bufs=40) as ptp,
            tc.tile_pool(name="osb", bufs=6) as osbp,
            tc.tile_pool(name="oout", bufs=3) as ooutp,
            tc.tile_pool(name="sc", bufs=2, space="PSUM") as pssc,     # 2x2 banks
            tc.tile_pool(name="avp", bufs=1, space="PSUM") as psav,    # 1 bank
            tc.tile_pool(name="mm", bufs=3, space="PSUM") as psmm,     # 3x1 bank
        ):
            # ---- persistent sbuf tiles ---------------------------------
            warm = singles.tile([128, 512], F16, name="warm")
            nc.vector.memset(warm[:], 0.0)

            iden = singles.tile([128, 128], F16, name="iden")
            nc.sync.dma_start(out=iden[:], in_=id_in[:])
            wk = singles.tile([128, 8 * DL], F16, name="wk")
            nc.sync.dma_start(out=wk[:], in_=wk_in[:])
            wq = singles.tile([128, 8 * DL], F16, name="wq")
            nc.sync.dma_start(out=wq[:], in_=wq_in[:])
            bq = singles.tile([128, 2], F32, name="bq")
            nc.sync.dma_start(out=bq[:], in_=bq_in[:])

            xt = singles.tile([128, 8 * T], F16, name="xt")
            xt3 = xt[:].rearrange("p (ci t) -> p ci t", ci=8)
            xsrc = x_in[:].rearrange("p (ci t) -> p ci t", ci=8)
            # x streamed in 8 slices of 256 tokens (one 128KB DMA each... 512KB)
            NSL = 8
            TSL = T // NSL
            for s in range(NSL):
                if s == 2:
                    wv = singles.tile([128, 8 * DL], F16, name="wv")
                    nc.sync.dma_start(out=wv[:], in_=wv_in[:])
                if s == 5:
                    wp = singles.tile([128, 2 * C], F16, name="wp")
                    nc.sync.dma_start(out=wp[:], in_=wp_in[:])
                nc.sync.dma_start(
                    out=xt3[:, :, s * TSL:(s + 1) * TSL],
                    in_=xsrc[:, :, s * TSL:(s + 1) * TSL],
                )

            qt = [singles.tile([128, T], F16, name=f"qt{m}") for m in range(2)]
            kt = [singles.tile([128, T], F16, name=f"kt{m}") for m in range(2)]
            v_sb = [singles.tile([128, HG * (DH + 1)], F16, name=f"v{tt}")
                    for tt in range(16)]
            for tt in range(16):
                nc.vector.memset(v_sb[tt][:, DH:HG * (DH + 1):DH + 1], 1.0)
            ot = [singles.tile([128, T], F16, name=f"ot{hp}") for hp in range(2)]

            # ---- PE warmup: chew through the pstate ramp while DMAs land
            for i in range(16):
                wps = psmm.tile([128, 512], F32, tag="mm", name=f"warm{i}")
                nc.tensor.matmul(wps[:], lhsT=warm[:, 0:128], rhs=warm[:],
                                 start=True, stop=True)

            # ---- building blocks ---------------------------------------
            def k_block(km, tb):
                """K projection for 512 tokens -> kt[km][:, tb*512:...]"""
                ps = psmm.tile([128, 512], F32, tag="mm", name=f"k{km}_{tb}")
                for half in range(2):
                    s = 2 * tb + half
                    for ci in range(8):
                        nc.tensor.matmul(
                            ps[:, half * 256:(half + 1) * 256],
                            lhsT=wk[:, ci * 256 + km * 128: ci * 256 + (km + 1) * 128],
                            rhs=xt3[:, ci, s * 256:(s + 1) * 256],
                            start=(ci == 0),
                            stop=(ci == 7),
                        )
                nc.vector.tensor_copy(kt[km][:, tb * 512:(tb + 1) * 512], ps[:])

            def q_block(qm, tb):
                ps = psmm.tile([128, 512], F32, tag="mm", name=f"q{qm}_{tb}")
                for half in range(2):
                    s = 2 * tb + half
                    for ci in range(8):
                        nc.tensor.matmul(
                            ps[:, half * 256:(half + 1) * 256],
                            lhsT=wq[:, ci * 256 + qm * 128: ci * 256 + (qm + 1) * 128],
                            rhs=xt3[:, ci, s * 256:(s + 1) * 256],
                            start=(ci == 0),
                            stop=(ci == 7),
                        )
                nc.vector.tensor_scalar_add(
                    qt[qm][:, tb * 512:(tb + 1) * 512], ps[:], bq[:, qm:qm + 1])

            def v_block(tt):
                """V projection for 128 tokens -> v_sb[tt] (65-col head blocks)"""
                ps = psmm.tile([128, 256], F32, tag="mm", name=f"v{tt}")
                for ci in range(8):
                    nc.tensor.matmul(
                        ps[:],
                        lhsT=xt3[:, ci, tt * 128:(tt + 1) * 128],
                        rhs=wv[:, ci * 256:(ci + 1) * 256],
                        start=(ci == 0),
                        stop=(ci == 7),
                    )
                dst = v_sb[tt][:].rearrange("p (h c) -> p h c", h=HG)[:, :, 0:DH]
                src = ps[:].rearrange("p (h c) -> p h c", h=HG)
                nc.vector.tensor_copy(dst, src)

            # scores tile p of unit (h, qb): key chunks 2p,2p+1 x 512 queries
            pt_tiles = {}

            def sc_tile(h, qb, p):
                km = h // 2
                row = (h % 2) * 64
                ps = pssc.tile([128, 1024], F32, tag="sc", name=f"s{h}_{qb}_{p}")
                for half in range(2):
                    st = 2 * p + half
                    nc.tensor.matmul(
                        ps[:, half * 512:(half + 1) * 512],
                        lhsT=kt[km][row:row + 64, st * 128:(st + 1) * 128],
                        rhs=qt[km][row:row + 64, qb * 512:(qb + 1) * 512],
                        start=True,
                        stop=True,
                    )
                pt = ptp.tile([128, 1024], F16, tag="pt", name=f"p{h}_{qb}_{p}")
                nc.scalar.activation(pt[:], ps[:], Exp, scale=SCALE)
                pt_tiles[(h, qb, p)] = pt

            osb_tiles = {}
            # one PSUM bank holds 4 rotating 65-col AV slots
            av_all = psav.tile([128, 4 * (DH + 1)], F32, name="av_all")
            av_ctr = [0]

            def av_group(h, qb, g):
                """AV^T for queries qtile=qb*4+g of head h -> divide into osb."""
                hp, col = h // 2, (h % 2) * 64
                slot = av_ctr[0] % 4
                av_ctr[0] += 1
                av = av_all[:, slot * (DH + 1):(slot + 1) * (DH + 1)]
                for st in range(16):
                    ptk = pt_tiles[(h, qb, st // 2)]
                    nc.tensor.matmul(
                        av[:],
                        lhsT=ptk[:, (st % 2) * 512 + g * 128:
                                 (st % 2) * 512 + (g + 1) * 128],
                        rhs=v_sb[st][:, h * (DH + 1):(h + 1) * (DH + 1)],
                        start=(st == 0),
                        stop=(st == 15),
                    )
                key = (hp, qb, g)
                if key not in osb_tiles:
                    osb_tiles[key] = osbp.tile([128, 128], F16, tag="osb",
                                               name=f"o{hp}_{qb}_{g}")
                rec = osbp.tile([128, 1], F32, tag="rec", bufs=4,
                                name=f"r{h}_{qb}_{g}")
                nc.vector.reciprocal(rec[:], av[:, DH:DH + 1])
                nc.vector.tensor_scalar_mul(
                    osb_tiles[key][:, col:col + 64], av[:, 0:DH], rec[:, 0:1])

            def transpose_hp(hp, qb):
                """osb pair tiles (4 qtiles) -> ot[hp][:, qb*512:...]"""
                ps = psmm.tile([128, 512], F16, tag="mm", name=f"t{hp}_{qb}")
                for g in range(4):
                    nc.tensor.transpose(
                        ps[:, g * 128:(g + 1) * 128],
                        osb_tiles[(hp, qb, g)][:],
                        iden[:],
                    )
                nc.vector.tensor_copy(ot[hp][:, qb * 512:(qb + 1) * 512], ps[:])

            def proj_tile(tt):
                o_out = ooutp.tile([128, C], F16, tag="oout", name=f"oo{tt}")
                for nb in range(2):
                    ps = psmm.tile([128, 512], F32, tag="mm", name=f"pr{tt}_{nb}")
                    for hp in range(2):
                        nc.tensor.matmul(
                            ps[:],
                            lhsT=ot[hp][:, tt * 128:(tt + 1) * 128],
                            rhs=wp[:, hp * C + nb * 512: hp * C + (nb + 1) * 512],
                            start=(hp == 0),
                            stop=(hp == 1),
                        )
                    nc.vector.tensor_copy(o_out[:, nb * 512:(nb + 1) * 512], ps[:])
                nc.sync.dma_start(out=out[tt * 128:(tt + 1) * 128, :], in_=o_out[:])

            # ---- intro: stream QKV per 512-token block, scores for qb0 --
            for tb in range(4):
                k_block(0, tb)
                if tb == 0:
                    q_block(0, 0)
                sc_tile(0, 0, 2 * tb)
                sc_tile(0, 0, 2 * tb + 1)
                k_block(1, tb)
                if tb == 0:
                    q_block(1, 0)
                sc_tile(1, 0, 2 * tb)
                sc_tile(1, 0, 2 * tb + 1)
                if tb == 2:
                    q_block(0, 1)
                sc_tile(2, 0, 2 * tb)
                sc_tile(2, 0, 2 * tb + 1)
                if tb == 3:
                    q_block(1, 1)
                sc_tile(3, 0, 2 * tb)
                sc_tile(3, 0, 2 * tb + 1)
                for tt in range(4 * tb, 4 * tb + 4):
                    v_block(tt)

            # ---- steady state: units u = qb*4 + h, AV lags by 2 units ---
            # per-unit fill work (Q blocks for later qbs, transposes, proj)
            def fills_for(u):
                """List of thunks to interleave into unit u's scores."""
                fl = []
                qb, h = u // 4, u % 4
                # Q blocks for qb+1 emitted during units of qb
                if 4 <= u <= 15:
                    nqb = qb + 1
                    if nqb <= 3:
                        if h == 0:
                            fl.append(lambda: q_block(0, nqb))
                        elif h == 1:
                            fl.append(lambda: q_block(1, nqb))
                return fl

            done_av = set()

            def emit_unit_scores(u, av_u, scores=True):
                """scores of unit u interleaved with AV groups of unit av_u"""
                qb, h = u // 4, u % 4
                fl = fills_for(u)
                for p in range(8):
                    if scores:
                        sc_tile(h, qb, p)
                    if av_u is not None and 2 <= p <= 5:
                        g = p - 2
                        av_group(av_u % 4, av_u // 4, g)
                        if g == 3:
                            done_av.add(av_u)
                            maybe_posts(av_u)
                    elif p == 6 and fl:
                        fl.pop(0)()
                if av_u is not None and av_u not in done_av:
                    for g in range(4):
                        av_group(av_u % 4, av_u // 4, g)
                    done_av.add(av_u)
                    maybe_posts(av_u)

            def maybe_posts(av_u):
                """After AV of unit av_u: transposes / proj as they unlock."""
                qb, h = av_u // 4, av_u % 4
                if h == 1:
                    transpose_hp(0, qb)
                elif h == 3:
                    transpose_hp(1, qb)
                    for tt in range(4 * qb, 4 * qb + 4):
                        proj_tile(tt)

            # units 4..15 scores; AV lag 2 (av of u-2 inside unit u).
            # u=2,3 emit no scores (intro covered qb0) but run AV of u0, u1.
            for u in range(2, 16):
                emit_unit_scores(u, u - 2, scores=(u >= 4))
            # drain: AV for units 14, 15
            for av_u in (14, 15):
                for g in range(4):
                    av_group(av_u % 4, av_u // 4, g)
                done_av.add(av_u)
                maybe_posts(av_u)

    nc.compile()
    return nc


_CACHE = {}


def _get_nc():
    if "nc" not in _CACHE:
        _CACHE["nc"] = build_bass()
    return _CACHE["nc"]


def _pack8(w):
    """[1024, n] -> [128, 8*n] with [p, ci*n+j] = w[ci*128+p, j]"""
    n = w.shape[1]
    return np.ascontiguousarray(
        w.reshape(8, 128, n).transpose(1, 0, 2).reshape(128, 8 * n))


def make_in_maps(x, w_qkv, b_qkv, w_proj):
    iden = np.eye(128, dtype=np.float16)
    in_maps = []
    for core in range(N_CORES):
        b = core // 4
        hg = core % 4
        cs = slice(hg * DL, (hg + 1) * DL)
        wq = w_qkv[:, 0 * C:1 * C][:, cs].astype(np.float16)
        wk = w_qkv[:, 1 * C:2 * C][:, cs].astype(np.float16)
        wv = w_qkv[:, 2 * C:3 * C][:, cs].astype(np.float16)
        bq = b_qkv[0 * C:1 * C][cs].astype(np.float32)
        xT = np.ascontiguousarray(x[b].T).astype(np.float16)   # [C, T]
        wp2 = w_proj[cs, :].astype(np.float16)                 # [256, 1024]
        wp_pack = np.ascontiguousarray(
            wp2.reshape(2, 128, C).transpose(1, 0, 2).reshape(128, 2 * C))
        in_maps.append({
            "x_pack": _pack8(xT),
            "wk_pack": _pack8(wk),
            "wq_pack": _pack8(wq),
            "wv_pack": _pack8(wv),
            "wp_pack": wp_pack,
            "b_q": np.stack([bq[0:128], bq[128:256]], axis=1),
            "iden16": iden,
        })
    return in_maps


def kernel(x, w_qkv, b_qkv, w_proj, b_proj, **runner_kwargs):
    x = np.asarray(x, dtype=np.float32)
    w_qkv = np.asarray(w_qkv, dtype=np.float32)
    b_qkv = np.asarray(b_qkv, dtype=np.float32)
    w_proj = np.asarray(w_proj, dtype=np.float32)
    b_proj = np.asarray(b_proj, dtype=np.float32)

    nc = _get_nc()
    in_maps = make_in_maps(x, w_qkv, b_qkv, w_proj)
    res = run_bass_kernel_spmd(nc, in_maps, list(range(N_CORES)), **runner_kwargs)
    parts = [res.results[i]["out_partial"] for i in range(N_CORES)]
    # fold V bias through the projection; K bias is softmax-invariant
    b_eff = b_proj + b_qkv[2 * C:3 * C].astype(np.float64) @ w_proj.astype(np.float64)
    outv = np.zeros((B, T, C), dtype=np.float32)
    for b in range(B):
        for hg in range(4):
            outv[b] += parts[4 * b + hg].astype(np.float32)
        outv[b] += b_eff.astype(np.float32)[None, :]
    if runner_kwargs:
        return outv, res
    return outv


if __name__ == "__main__":
    import reference

    inputs = reference.setup_inputs()
    inputs = {k: np.asarray(v) for k, v in inputs.items()}
    got = kernel(**inputs)
    want = np.asarray(reference.reference(**inputs))
    err = np.abs(got - want).max() / np.abs(want).max()
    print("rel err:", err)


# revision 15
# speedup vs baseline: 1.2752x; 1.0064x over previous
"""Multi-head self-attention Trainium2 kernel, sharded over 8 NeuronCores.

Sharding: core = (batch, head_group): 2 batches x 4 head-groups (4 heads each).
Each core computes qkv for its batch restricted to its heads, full-sequence
attention for those heads, and a row-parallel slice of the output projection,
producing a partial [T, C] output (fp16). Host: out[b] = sum of the 4
head-group partials + b_eff where b_eff folds b_proj and the V bias.

v2 design notes (all relative to the fp32/on-chip-transpose baseline):
  - x is transposed, packed and cast to fp16 on the host; no on-chip
    transposes or x^T copies are needed.
  - K bias is dropped entirely (softmax is invariant to per-query constants,
    and q.bk is per-query); V bias is folded into b_proj on the host
    (sum_s w_s = 1); only the Q bias is applied on-chip.
  - AV is computed transposed: out[q, d] = sum_s P[s,q] V[s,d] with
    ap_size=65 per chunk matmul, which halves the PE cost of AV and makes
    the softmax divide a single per-partition tensor_scalar divide.
  - The softmax denominator comes from a ones-column appended per head in
    the V tile (memset once).
  - Everything on the PE runs fp16 (1.0 cycles/row); fp8 was measured to
    break the 2e-2 gate (diffuse attention preserves per-key noise).
"""

import math
import sys

import numpy as np

sys.path.insert(0, "/opt/trn_rl_repo")

import concourse.bacc as bacc
import concourse.bass as bass
import concourse.tile as tile
from concourse import mybir
from concourse.bass_utils import run_bass_kernel_spmd

B, T, C = 2, 2048, 1024
NH, DH = 16, 64
HG = 4                  # heads per core
DL = HG * DH            # 256 local head dims
N_CORES = 8

F32 = mybir.dt.float32
F16 = mybir.dt.float16

SCALE = 1.0 / math.sqrt(DH)
Exp = mybir.ActivationFunctionType.Exp


def build_bass():
    nc = bacc.Bacc("TRN2", target_bir_lowering=False, debug=False)

    # host-packed params: [p, ci*w + j] = w[ci*128 + p, j]
    x_in = nc.declare_dram_parameter("x_pack", [128, 8 * T], F16, isOutput=False)
    wk_in = nc.declare_dram_parameter("wk_pack", [128, 8 * DL], F16, isOutput=False)
    wq_in = nc.declare_dram_parameter("wq_pack", [128, 8 * DL], F16, isOutput=False)
    wv_in = nc.declare_dram_parameter("wv_pack", [128, 8 * DL], F16, isOutput=False)
    wp_in = nc.declare_dram_parameter("wp_pack", [128, 2 * C], F16, isOutput=False)
    bq_in = nc.declare_dram_parameter("b_q", [128, 2], F32, isOutput=False)
    id_in = nc.declare_dram_parameter("iden16", [128, 128], F16, isOutput=False)
    out = nc.declare_dram_parameter("out_partial", [T, C], F16, isOutput=True)

    with tile.TileContext(nc) as tc:
        with (
            tc.tile_pool(name="singles", bufs=1) as singles,
            tc.tile_pool(name="pt", bufs=44) as ptp,
            tc.tile_pool(name="osb", bufs=6) as osbp,
            tc.tile_pool(name="oout", bufs=3) as ooutp,
            tc.tile_pool(name="sc", bufs=2, space="PSUM") as pssc,     # 2x2 banks
            tc.tile_pool(name="avp", bufs=1, space="PSUM") as psav,    # 1 bank
            tc.tile_pool(name="mm", bufs=3, space="PSUM") as psmm,     # 3x1 bank
        ):
            # ---- persistent sbuf tiles ---------------------------------
            warm = singles.tile([128, 512], F16, name="warm")
            nc.vector.memset(warm[:], 0.0)
            # pre-load the Exp activation table while DMAs are in flight
            warm_exp = singles.tile([128, 1], F16, name="warm_exp")
            nc.scalar.activation(warm_exp[:], warm[:, 0:1], Exp, scale=SCALE)

            xt = singles.tile([128, 8 * T], F16, name="xt")
            xt3 = xt[:].rearrange("p (ci t) -> p ci t", ci=8)
            xsrc = x_in[:].rearrange("p (ci t) -> p ci t", ci=8)
            NSL = 8
            TSL = T // NSL

            def x_slice(s):
                nc.sync.dma_start(
                    out=xt3[:, :, s * TSL:(s + 1) * TSL],
                    in_=xsrc[:, :, s * TSL:(s + 1) * TSL],
                )

            # DMA order tuned so the first score tile unblocks earliest
            wk = singles.tile([128, 8 * DL], F16, name="wk")
            nc.sync.dma_start(out=wk[:], in_=wk_in[:])
            x_slice(0)
            wq = singles.tile([128, 8 * DL], F16, name="wq")
            nc.sync.dma_start(out=wq[:], in_=wq_in[:])
            bq = singles.tile([128, 2], F32, name="bq")
            nc.sync.dma_start(out=bq[:], in_=bq_in[:])
            x_slice(1)
            x_slice(2)
            x_slice(3)
            wv = singles.tile([128, 8 * DL], F16, name="wv")
            nc.sync.dma_start(out=wv[:], in_=wv_in[:])
            x_slice(4)
            x_slice(5)
            x_slice(6)
            x_slice(7)
            wp = singles.tile([128, 2 * C], F16, name="wp")
            nc.sync.dma_start(out=wp[:], in_=wp_in[:])
            iden = singles.tile([128, 128], F16, name="iden")
            nc.sync.dma_start(out=iden[:], in_=id_in[:])

            qt = [singles.tile([128, T], F16, name=f"qt{m}") for m in range(2)]
            kt = [singles.tile([128, T], F16, name=f"kt{m}") for m in range(2)]
            v_sb = [singles.tile([128, HG * (DH + 1)], F16, name=f"v{tt}")
                    for tt in range(16)]
            for tt in range(16):
                nc.vector.memset(v_sb[tt][:, DH:HG * (DH + 1):DH + 1], 1.0)
            ot = [singles.tile([128, T], F16, name=f"ot{hp}") for hp in range(2)]

            # ---- PE warmup: chew through the pstate ramp while DMAs land
            for i in range(8):
                wps = psmm.tile([128, 512], F32, tag="mm", name=f"warm{i}")
                nc.tensor.matmul(wps[:], lhsT=warm[:, 0:128], rhs=warm[:],
                                 start=True, stop=True)

            # ---- building blocks ---------------------------------------
            def k_block(km, tb):
                """K projection for 512 tokens -> kt[km][:, tb*512:...]"""
                ps = psmm.tile([128, 512], F32, tag="mm", name=f"k{km}_{tb}")
                for half in range(2):
                    s = 2 * tb + half
                    for ci in range(8):
                        nc.tensor.matmul(
                            ps[:, half * 256:(half + 1) * 256],
                            lhsT=wk[:, ci * 256 + km * 128: ci * 256 + (km + 1) * 128],
                            rhs=xt3[:, ci, s * 256:(s + 1) * 256],
                            start=(ci == 0),
                            stop=(ci == 7),
                        )
                nc.vector.tensor_copy(kt[km][:, tb * 512:(tb + 1) * 512], ps[:])

            def q_block(qm, tb):
                ps = psmm.tile([128, 512], F32, tag="mm", name=f"q{qm}_{tb}")
                for half in range(2):
                    s = 2 * tb + half
                    for ci in range(8):
                        nc.tensor.matmul(
                            ps[:, half * 256:(half + 1) * 256],
                            lhsT=wq[:, ci * 256 + qm * 128: ci * 256 + (qm + 1) * 128],
                            rhs=xt3[:, ci, s * 256:(s + 1) * 256],
                            start=(ci == 0),
                            stop=(ci == 7),
                        )
                nc.vector.tensor_scalar_add(
                    qt[qm][:, tb * 512:(tb + 1) * 512], ps[:], bq[:, qm:qm + 1])

            def v_block(tt):
                """V projection for 128 tokens -> v_sb[tt] (65-col head blocks)"""
                ps = psmm.tile([128, 256], F32, tag="mm", name=f"v{tt}")
                for ci in range(8):
                    nc.tensor.matmul(
                        ps[:],
                        lhsT=xt3[:, ci, tt * 128:(tt + 1) * 128],
                        rhs=wv[:, ci * 256:(ci + 1) * 256],
                        start=(ci == 0),
                        stop=(ci == 7),
                    )
                dst = v_sb[tt][:].rearrange("p (h c) -> p h c", h=HG)[:, :, 0:DH]
                src = ps[:].rearrange("p (h c) -> p h c", h=HG)
                nc.vector.tensor_copy(dst, src)

            # scores tile p of unit (h, qb): key chunks 2p,2p+1 x 512 queries
            pt_tiles = {}

            def sc_tile(h, qb, p):
                km = h // 2
                row = (h % 2) * 64
                ps = pssc.tile([128, 1024], F32, tag="sc", name=f"s{h}_{qb}_{p}")
                for half in range(2):
                    st = 2 * p + half
                    nc.tensor.matmul(
                        ps[:, half * 512:(half + 1) * 512],
                        lhsT=kt[km][row:row + 64, st * 128:(st + 1) * 128],
                        rhs=qt[km][row:row + 64, qb * 512:(qb + 1) * 512],
                        start=True,
                        stop=True,
                    )
                pt = ptp.tile([128, 1024], F16, tag="pt", name=f"p{h}_{qb}_{p}")
                nc.scalar.activation(pt[:], ps[:], Exp, scale=SCALE)
                pt_tiles[(h, qb, p)] = pt

            osb_tiles = {}
            # one PSUM bank holds 4 rotating 65-col AV slots
            av_all = psav.tile([128, 4 * (DH + 1)], F32, name="av_all")
            av_ctr = [0]

            def av_group(h, qb, g):
                """AV^T for queries qtile=qb*4+g of head h -> divide into osb."""
                hp, col = h // 2, (h % 2) * 64
                slot = av_ctr[0] % 4
                av_ctr[0] += 1
                av = av_all[:, slot * (DH + 1):(slot + 1) * (DH + 1)]
                for st in range(16):
                    ptk = pt_tiles[(h, qb, st // 2)]
                    nc.tensor.matmul(
                        av[:],
                        lhsT=ptk[:, (st % 2) * 512 + g * 128:
                                 (st % 2) * 512 + (g + 1) * 128],
                        rhs=v_sb[st][:, h * (DH + 1):(h + 1) * (DH + 1)],
                        start=(st == 0),
                        stop=(st == 15),
                    )
                key = (hp, qb, g)
                if key not in osb_tiles:
                    osb_tiles[key] = osbp.tile([128, 128], F16, tag="osb",
                                               name=f"o{hp}_{qb}_{g}")
                rec = osbp.tile([128, 1], F32, tag="rec", bufs=4,
                                name=f"r{h}_{qb}_{g}")
                nc.vector.reciprocal(rec[:], av[:, DH:DH + 1])
                nc.vector.tensor_scalar_mul(
                    osb_tiles[key][:, col:col + 64], av[:, 0:DH], rec[:, 0:1])

            def transpose_hp(hp, qb):
                """osb pair tiles (4 qtiles) -> ot[hp][:, qb*512:...]"""
                ps = psmm.tile([128, 512], F16, tag="mm", name=f"t{hp}_{qb}")
                for g in range(4):
                    nc.tensor.transpose(
                        ps[:, g * 128:(g + 1) * 128],
                        osb_tiles[(hp, qb, g)][:],
                        iden[:],
                    )
                nc.vector.tensor_copy(ot[hp][:, qb * 512:(qb + 1) * 512], ps[:])

            Copy = mybir.ActivationFunctionType.Copy

            def proj_tile(tt, use_act=False):
                o_out = ooutp.tile([128, C], F16, tag="oout", name=f"oo{tt}")
                for nb in range(2):
                    ps = psmm.tile([128, 512], F32, tag="mm", name=f"pr{tt}_{nb}")
                    for hp in range(2):
                        nc.tensor.matmul(
                            ps[:],
                            lhsT=ot[hp][:, tt * 128:(tt + 1) * 128],
                            rhs=wp[:, hp * C + nb * 512: hp * C + (nb + 1) * 512],
                            start=(hp == 0),
                            stop=(hp == 1),
                        )
                    dst = o_out[:, nb * 512:(nb + 1) * 512]
                    if use_act:
                        # tail: ACT is idle after the last exp, DVE is not
                        nc.scalar.activation(dst, ps[:], Copy)
                    else:
                        nc.vector.tensor_copy(dst, ps[:])
                nc.sync.dma_start(out=out[tt * 128:(tt + 1) * 128, :], in_=o_out[:])

            # ---- intro: stream QKV per 512-token block, scores for qb0 --
            # V blocks interleaved between score tiles to keep ACT fed
            for tb in range(4):
                k_block(0, tb)
                if tb == 0:
                    q_block(0, 0)
                sc_tile(0, 0, 2 * tb)
                v_block(4 * tb + 0)
                sc_tile(0, 0, 2 * tb + 1)
                k_block(1, tb)
                if tb == 0:
                    q_block(1, 0)
                sc_tile(1, 0, 2 * tb)
                v_block(4 * tb + 1)
                sc_tile(1, 0, 2 * tb + 1)
                if tb == 2:
                    q_block(0, 1)
                sc_tile(2, 0, 2 * tb)
                v_block(4 * tb + 2)
                sc_tile(2, 0, 2 * tb + 1)
                if tb == 3:
                    q_block(1, 1)
                sc_tile(3, 0, 2 * tb)
                v_block(4 * tb + 3)
                sc_tile(3, 0, 2 * tb + 1)

            # ---- steady state: units u = qb*4 + h, AV lags by 2 units ---
            # per-unit fill work (Q blocks for later qbs, transposes, proj)
            def fills_for(u):
                """List of thunks to interleave into unit u's scores."""
                fl = []
                qb, h = u // 4, u % 4
                # Q blocks for qb+1 emitted during units of qb
                if 4 <= u <= 15:
                    nqb = qb + 1
                    if nqb <= 3:
                        if h == 0:
                            fl.append(lambda: q_block(0, nqb))
                        elif h == 1:
                            fl.append(lambda: q_block(1, nqb))
                return fl

            done_av = set()

            def full_av(av_u):
                for g in range(4):
                    av_group(av_u % 4, av_u // 4, g)
                done_av.add(av_u)
                maybe_posts(av_u)

            def emit_unit_scores(u, av_us):
                """scores of unit u interleaved with AV groups of av_us"""
                qb, h = u // 4, u % 4
                fl = fills_for(u)
                # slot plan: p2..p5 -> first AV's 4 groups; p6,p7(+end) -> rest
                first = av_us[0] if av_us else None
                for p in range(8):
                    sc_tile(h, qb, p)
                    if first is not None and 2 <= p <= 5:
                        g = p - 2
                        av_group(first % 4, first // 4, g)
                        if g == 3:
                            done_av.add(first)
                            maybe_posts(first)
                    elif p == 6 and fl:
                        fl.pop(0)()
                for av_u in av_us[1:]:
                    full_av(av_u)

            def maybe_posts(av_u):
                """After AV of unit av_u: transposes / proj as they unlock."""
                qb, h = av_u // 4, av_u % 4
                tail = qb == 3
                if h == 1:
                    transpose_hp(0, qb)
                elif h == 3:
                    transpose_hp(1, qb)
                    for tt in range(4 * qb, 4 * qb + 4):
                        proj_tile(tt, use_act=tail)

            # AV assignment per unit: u4/u5 double up (covers the intro's qb0
            # units), u15 also runs u14's AV so the drain is only AV(u15).
            av_plan = {4: [0, 1], 5: [2, 3], 15: [13, 14]}
            for u in range(4, 16):
                emit_unit_scores(u, av_plan.get(u, [u - 2]))
            full_av(15)

    nc.compile()
    return nc


_CACHE = {}


def _get_nc():
    if "nc" not in _CACHE:
        _CACHE["nc"] = build_bass()
    return _CACHE["nc"]


def _pack8(w):
    """[1024, n] -> [128, 8*n] with [p, ci*n+j] = w[ci*128+p, j]"""
    n = w.shape[1]
    return np.ascontiguousarray(
        w.reshape(8, 128, n).transpose(1, 0, 2).reshape(128, 8 * n))


def make_in_maps(x, w_qkv, b_qkv, w_proj):
    iden = np.eye(128, dtype=np.float16)
    in_maps = []
    for core in range(N_CORES):
        b = core // 4
        hg = core % 4
        cs = slice(hg * DL, (hg + 1) * DL)
        wq = w_qkv[:, 0 * C:1 * C][:, cs].astype(np.float16)
        wk = w_qkv[:, 1 * C:2 * C][:, cs].astype(np.float16)
        wv = w_qkv[:, 2 * C:3 * C][:, cs].astype(np.float16)
        bq = b_qkv[0 * C:1 * C][cs].astype(np.float32)
        xT = np.ascontiguousarray(x[b].T).astype(np.float16)   # [C, T]
        wp2 = w_proj[cs, :].astype(np.float16)                 # [256, 1024]
        wp_pack = np.ascontiguousarray(
            wp2.reshape(2, 128, C).transpose(1, 0, 2).reshape(128, 2 * C))
        in_maps.append({
            "x_pack": _pack8(xT),
            "wk_pack": _pack8(wk),
            "wq_pack": _pack8(wq),
            "wv_pack": _pack8(wv),
            "wp_pack": wp_pack,
            "b_q": np.stack([bq[0:128], bq[128:256]], axis=1),
            "iden16": iden,
        })
    return in_maps


def kernel(x, w_qkv, b_qkv, w_proj, b_proj, **runner_kwargs):
    x = np.asarray(x, dtype=np.float32)
    w_qkv = np.asarray(w_qkv, dtype=np.float32)
    b_qkv = np.asarray(b_qkv, dtype=np.float32)
    w_proj = np.asarray(w_proj, dtype=np.float32)
    b_proj = np.asarray(b_proj, dtype=np.float32)

    nc = _get_nc()
    in_maps = make_in_maps(x, w_qkv, b_qkv, w_proj)
    res = run_bass_kernel_spmd(nc, in_maps, list(range(N_CORES)), **runner_kwargs)
    parts = [res.results[i]["out_partial"] for i in range(N_CORES)]
    # fold V bias through the projection; K bias is softmax-invariant
    b_eff = b_proj + b_qkv[2 * C:3 * C].astype(np.float64) @ w_proj.astype(np.float64)
    outv = np.zeros((B, T, C), dtype=np.float32)
    for b in range(B):
        for hg in range(4):
            outv[b] += parts[4 * b + hg].astype(np.float32)
        outv[b] += b_eff.astype(np.float32)[None, :]
    if runner_kwargs:
        return outv, res
    return outv


if __name__ == "__main__":
    import reference

    inputs = reference.setup_inputs()
    inputs = {k: np.asarray(v) for k, v in inputs.items()}
    got = kernel(**inputs)
    want = np.asarray(reference.reference(**inputs))
    err = np.abs(got - want).max() / np.abs(want).max()
    print("rel err:", err)


# revision 17
# speedup vs baseline: 1.3332x; 1.0455x over previous
"""Multi-head self-attention Trainium2 kernel, sharded over 8 NeuronCores.

Sharding: core = (batch, head_group): 2 batches x 4 head-groups (4 heads each).
Each core computes qkv for its batch restricted to its heads, full-sequence
attention for those heads, and a row-parallel slice of the output projection,
producing a partial [T, C] output (fp16). Host: out[b] = sum of the 4
head-group partials + b_eff where b_eff folds b_proj and the V bias.

v2 design notes (all relative to the fp32/on-chip-transpose baseline):
  - x is transposed, packed and cast to fp16 on the host; no on-chip
    transposes or x^T copies are needed.
  - K bias is dropped entirely (softmax is invariant to per-query constants,
    and q.bk is per-query); V bias is folded into b_proj on the host
    (sum_s w_s = 1); only the Q bias is applied on-chip.
  - AV is computed transposed: out[q, d] = sum_s P[s,q] V[s,d] with
    ap_size=65 per chunk matmul, which halves the PE cost of AV and makes
    the softmax divide a single per-partition tensor_scalar divide.
  - The softmax denominator comes from a ones-column appended per head in
    the V tile (memset once).
  - Everything on the PE runs fp16 (1.0 cycles/row); fp8 was measured to
    break the 2e-2 gate (diffuse attention preserves per-key noise).
"""

import math
import sys

import numpy as np

sys.path.insert(0, "/opt/trn_rl_repo")

import concourse.bacc as bacc
import concourse.bass as bass
import concourse.tile as tile
from concourse import mybir
from concourse.bass_utils import run_bass_kernel_spmd

B, T, C = 2, 2048, 1024
NH, DH = 16, 64
HG = 4                  # heads per core
DL = HG * DH            # 256 local head dims
N_CORES = 8

F32 = mybir.dt.float32
F16 = mybir.dt.float16

SCALE = 1.0 / math.sqrt(DH)
Exp = mybir.ActivationFunctionType.Exp


def build_bass():
    nc = bacc.Bacc("TRN2", target_bir_lowering=False, debug=False)

    # host-packed params: [p, ci*w + j] = w[ci*128 + p, j]
    x_in = nc.declare_dram_parameter("x_pack", [128, 8 * T], F16, isOutput=False)
    wk_in = nc.declare_dram_parameter("wk_pack", [128, 8 * DL], F16, isOutput=False)
    wq_in = nc.declare_dram_parameter("wq_pack", [128, 8 * DL], F16, isOutput=False)
    wv_in = nc.declare_dram_parameter("wv_pack", [128, 8 * DL], F16, isOutput=False)
    wp_in = nc.declare_dram_parameter("wp_pack", [128, 2 * C], F16, isOutput=False)
    bq_in = nc.declare_dram_parameter("b_q", [128, 2], F32, isOutput=False)
    id_in = nc.declare_dram_parameter("iden16", [128, 128], F16, isOutput=False)
    out = nc.declare_dram_parameter("out_partial", [T, C], F16, isOutput=True)

    with tile.TileContext(nc) as tc:
        with (
            tc.tile_pool(name="singles", bufs=1) as singles,
            tc.tile_pool(name="pt", bufs=44) as ptp,
            tc.tile_pool(name="osb", bufs=6) as osbp,
            tc.tile_pool(name="oout", bufs=3) as ooutp,
            tc.tile_pool(name="sc", bufs=2, space="PSUM") as pssc,     # 2x2 banks
            tc.tile_pool(name="avp", bufs=1, space="PSUM") as psav,    # 1 bank
            tc.tile_pool(name="mm", bufs=3, space="PSUM") as psmm,     # 3x1 bank
        ):
            # ---- persistent sbuf tiles ---------------------------------
            warm = singles.tile([128, 512], F16, name="warm")
            nc.vector.memset(warm[:], 0.0)
            # pre-load the Exp activation table while DMAs are in flight
            warm_exp = singles.tile([128, 1], F16, name="warm_exp")
            nc.scalar.activation(warm_exp[:], warm[:, 0:1], Exp, scale=SCALE)

            xt = singles.tile([128, 8 * T], F16, name="xt")
            xt3 = xt[:].rearrange("p (ci t) -> p ci t", ci=8)
            xsrc = x_in[:].rearrange("p (ci t) -> p ci t", ci=8)
            NSL = 8
            TSL = T // NSL

            def x_slice(s):
                nc.sync.dma_start(
                    out=xt3[:, :, s * TSL:(s + 1) * TSL],
                    in_=xsrc[:, :, s * TSL:(s + 1) * TSL],
                )

            # DMA order tuned so the first score tile unblocks earliest
            wk = singles.tile([128, 8 * DL], F16, name="wk")
            nc.sync.dma_start(out=wk[:], in_=wk_in[:])
            x_slice(0)
            wq = singles.tile([128, 8 * DL], F16, name="wq")
            nc.sync.dma_start(out=wq[:], in_=wq_in[:])
            bq = singles.tile([128, 2], F32, name="bq")
            nc.sync.dma_start(out=bq[:], in_=bq_in[:])
            x_slice(1)
            x_slice(2)
            x_slice(3)
            wv = singles.tile([128, 8 * DL], F16, name="wv")
            nc.sync.dma_start(out=wv[:], in_=wv_in[:])
            x_slice(4)
            x_slice(5)
            x_slice(6)
            x_slice(7)
            wp = singles.tile([128, 2 * C], F16, name="wp")
            nc.sync.dma_start(out=wp[:], in_=wp_in[:])
            iden = singles.tile([128, 128], F16, name="iden")
            nc.sync.dma_start(out=iden[:], in_=id_in[:])

            qt = [singles.tile([128, T], F16, name=f"qt{m}") for m in range(2)]
            kt = [singles.tile([128, T], F16, name=f"kt{m}") for m in range(2)]
            v_sb = [singles.tile([128, HG * (DH + 1)], F16, name=f"v{tt}")
                    for tt in range(16)]
            for tt in range(16):
                nc.vector.memset(v_sb[tt][:, DH:HG * (DH + 1):DH + 1], 1.0)
            ot = [singles.tile([128, T], F16, name=f"ot{hp}") for hp in range(2)]

            # ---- PE warmup: chew through the pstate ramp while DMAs land
            for i in range(8):
                wps = psmm.tile([128, 512], F32, tag="mm", name=f"warm{i}")
                nc.tensor.matmul(wps[:], lhsT=warm[:, 0:128], rhs=warm[:],
                                 start=True, stop=True)

            # ---- building blocks ---------------------------------------
            def k_block(km, tb):
                """K projection for 512 tokens -> kt[km][:, tb*512:...]

                Two half tiles so the psum->sbuf copy of the first 256
                tokens overlaps the second half's matmuls."""
                for half in range(2):
                    s = 2 * tb + half
                    ps = psmm.tile([128, 256], F32, tag="mm", name=f"k{km}_{s}")
                    for ci in range(8):
                        nc.tensor.matmul(
                            ps[:],
                            lhsT=wk[:, ci * 256 + km * 128: ci * 256 + (km + 1) * 128],
                            rhs=xt3[:, ci, s * 256:(s + 1) * 256],
                            start=(ci == 0),
                            stop=(ci == 7),
                        )
                    nc.vector.tensor_copy(kt[km][:, s * 256:(s + 1) * 256], ps[:])

            def q_block(qm, tb):
                for half in range(2):
                    s = 2 * tb + half
                    ps = psmm.tile([128, 256], F32, tag="mm", name=f"q{qm}_{s}")
                    for ci in range(8):
                        nc.tensor.matmul(
                            ps[:],
                            lhsT=wq[:, ci * 256 + qm * 128: ci * 256 + (qm + 1) * 128],
                            rhs=xt3[:, ci, s * 256:(s + 1) * 256],
                            start=(ci == 0),
                            stop=(ci == 7),
                        )
                    nc.vector.tensor_scalar_add(
                        qt[qm][:, s * 256:(s + 1) * 256], ps[:], bq[:, qm:qm + 1])

            def v_block(tt):
                """V projection for 128 tokens -> v_sb[tt] (65-col head blocks)"""
                ps = psmm.tile([128, 256], F32, tag="mm", name=f"v{tt}")
                for ci in range(8):
                    nc.tensor.matmul(
                        ps[:],
                        lhsT=xt3[:, ci, tt * 128:(tt + 1) * 128],
                        rhs=wv[:, ci * 256:(ci + 1) * 256],
                        start=(ci == 0),
                        stop=(ci == 7),
                    )
                dst = v_sb[tt][:].rearrange("p (h c) -> p h c", h=HG)[:, :, 0:DH]
                src = ps[:].rearrange("p (h c) -> p h c", h=HG)
                nc.vector.tensor_copy(dst, src)

            # scores tile p of unit (h, qb): key chunks 2p,2p+1 x 512 queries
            pt_tiles = {}

            def sc_tile(h, qb, p):
                km = h // 2
                row = (h % 2) * 64
                ps = pssc.tile([128, 1024], F32, tag="sc", name=f"s{h}_{qb}_{p}")
                for half in range(2):
                    st = 2 * p + half
                    nc.tensor.matmul(
                        ps[:, half * 512:(half + 1) * 512],
                        lhsT=kt[km][row:row + 64, st * 128:(st + 1) * 128],
                        rhs=qt[km][row:row + 64, qb * 512:(qb + 1) * 512],
                        start=True,
                        stop=True,
                    )
                pt = ptp.tile([128, 1024], F16, tag="pt", name=f"p{h}_{qb}_{p}")
                nc.scalar.activation(pt[:], ps[:], Exp, scale=SCALE)
                pt_tiles[(h, qb, p)] = pt

            osb_tiles = {}
            # one PSUM bank holds 4 rotating 65-col AV slots
            av_all = psav.tile([128, 4 * (DH + 1)], F32, name="av_all")
            av_ctr = [0]

            def av_group(h, qb, g):
                """AV^T for queries qtile=qb*4+g of head h -> divide into osb."""
                hp, col = h // 2, (h % 2) * 64
                slot = av_ctr[0] % 4
                av_ctr[0] += 1
                av = av_all[:, slot * (DH + 1):(slot + 1) * (DH + 1)]
                for st in range(16):
                    ptk = pt_tiles[(h, qb, st // 2)]
                    nc.tensor.matmul(
                        av[:],
                        lhsT=ptk[:, (st % 2) * 512 + g * 128:
                                 (st % 2) * 512 + (g + 1) * 128],
                        rhs=v_sb[st][:, h * (DH + 1):(h + 1) * (DH + 1)],
                        start=(st == 0),
                        stop=(st == 15),
                    )
                key = (hp, qb, g)
                if key not in osb_tiles:
                    osb_tiles[key] = osbp.tile([128, 128], F16, tag="osb",
                                               name=f"o{hp}_{qb}_{g}")
                rec = osbp.tile([128, 1], F32, tag="rec", bufs=4,
                                name=f"r{h}_{qb}_{g}")
                nc.vector.reciprocal(rec[:], av[:, DH:DH + 1])
                nc.vector.tensor_scalar_mul(
                    osb_tiles[key][:, col:col + 64], av[:, 0:DH], rec[:, 0:1])

            def transpose_hp(hp, qb):
                """osb pair tiles (4 qtiles) -> ot[hp][:, qb*512:...]"""
                ps = psmm.tile([128, 512], F16, tag="mm", name=f"t{hp}_{qb}")
                for g in range(4):
                    nc.tensor.transpose(
                        ps[:, g * 128:(g + 1) * 128],
                        osb_tiles[(hp, qb, g)][:],
                        iden[:],
                    )
                nc.vector.tensor_copy(ot[hp][:, qb * 512:(qb + 1) * 512], ps[:])

            Copy = mybir.ActivationFunctionType.Copy

            def proj_tile(tt, use_act=False):
                o_out = ooutp.tile([128, C], F16, tag="oout", name=f"oo{tt}")
                for nb in range(2):
                    ps = psmm.tile([128, 512], F32, tag="mm", name=f"pr{tt}_{nb}")
                    for hp in range(2):
                        nc.tensor.matmul(
                            ps[:],
                            lhsT=ot[hp][:, tt * 128:(tt + 1) * 128],
                            rhs=wp[:, hp * C + nb * 512: hp * C + (nb + 1) * 512],
                            start=(hp == 0),
                            stop=(hp == 1),
                        )
                    dst = o_out[:, nb * 512:(nb + 1) * 512]
                    if use_act:
                        # tail: ACT is idle after the last exp, DVE is not
                        nc.scalar.activation(dst, ps[:], Copy)
                    else:
                        nc.vector.tensor_copy(dst, ps[:])
                nc.sync.dma_start(out=out[tt * 128:(tt + 1) * 128, :], in_=o_out[:])

            # ---- fill queue: transposes + proj consumed in spare PE slots
            fillq = []
            done_av = set()

            def maybe_posts(av_u):
                qb, h = av_u // 4, av_u % 4
                tail = qb == 3
                if h == 1:
                    fillq.append(lambda qb=qb: transpose_hp(0, qb))
                elif h == 3:
                    fillq.append(lambda qb=qb: transpose_hp(1, qb))
                    for tt in range(4 * qb, 4 * qb + 4):
                        fillq.append(
                            lambda tt=tt, tl=tail: proj_tile(tt, use_act=tl))

            def full_av(av_u):
                for g in range(4):
                    av_group(av_u % 4, av_u // 4, g)
                done_av.add(av_u)
                maybe_posts(av_u)

            # ---- intro: K + qb0 scores dominate; only 12 V blocks and the
            # qb0 Q blocks live here so ACT stays fed from the start.
            IV = {0: [0], 1: [1, 2, 3], 2: [4, 5, 6], 3: [7, 8, 9, 10, 11]}
            for tb in range(4):
                k_block(0, tb)
                if tb == 0:
                    q_block(0, 0)
                vq = list(IV[tb])
                sc_tile(0, 0, 2 * tb)
                if vq:
                    v_block(vq.pop(0))
                sc_tile(0, 0, 2 * tb + 1)
                k_block(1, tb)
                if tb == 0:
                    q_block(1, 0)
                sc_tile(1, 0, 2 * tb)
                if vq:
                    v_block(vq.pop(0))
                sc_tile(1, 0, 2 * tb + 1)
                sc_tile(2, 0, 2 * tb)
                if vq:
                    v_block(vq.pop(0))
                sc_tile(2, 0, 2 * tb + 1)
                sc_tile(3, 0, 2 * tb)
                if vq:
                    v_block(vq.pop(0))
                sc_tile(3, 0, 2 * tb + 1)
                while vq:
                    v_block(vq.pop(0))

            # ---- steady state: units u = qb*4 + h -----------------------
            pre_fills = {4: [lambda: q_block(0, 1)]}
            mid_fills = {5: [lambda: q_block(1, 1)],
                         6: [lambda: q_block(0, 2)],
                         7: [lambda: q_block(1, 2)],
                         8: [lambda: q_block(0, 3)],
                         9: [lambda: q_block(1, 3)]}
            unit_v = {4: [12, 13, 14, 15]}
            av_plan = {5: [0], 6: [1], 7: [2, 3], 8: [4, 5], 9: [6, 7],
                       10: [8], 11: [9], 12: [10], 13: [11], 14: [12],
                       15: [13, 14]}

            def emit_unit(u):
                qb, h = u // 4, u % 4
                for f in pre_fills.get(u, []):
                    f()
                avs = av_plan.get(u, [])
                first = avs[0] if avs else None
                extras = [lambda tt=tt: v_block(tt) for tt in unit_v.get(u, [])]
                extras += mid_fills.get(u, [])
                nfill = 2
                for p in range(8):
                    sc_tile(h, qb, p)
                    if first is not None and 2 <= p <= 5:
                        g = p - 2
                        av_group(first % 4, first // 4, g)
                        if g == 3:
                            done_av.add(first)
                            maybe_posts(first)
                    elif extras:
                        extras.pop(0)()
                    elif nfill and fillq:
                        nfill -= 1
                        fillq.pop(0)()
                while extras:
                    extras.pop(0)()
                for av_u in avs[1:]:
                    full_av(av_u)

            for u in range(4, 16):
                emit_unit(u)
            full_av(15)
            while fillq:
                fillq.pop(0)()

    nc.compile()
    return nc


_CACHE = {}


def _get_nc():
    if "nc" not in _CACHE:
        _CACHE["nc"] = build_bass()
    return _CACHE["nc"]


def _pack8(w):
    """[1024, n] -> [128, 8*n] with [p, ci*n+j] = w[ci*128+p, j]"""
    n = w.shape[1]
    return np.ascontiguousarray(
        w.reshape(8, 128, n).transpose(1, 0, 2).reshape(128, 8 * n))


def make_in_maps(x, w_qkv, b_qkv, w_proj):
    iden = np.eye(128, dtype=np.float16)
    in_maps = []
    for core in range(N_CORES):
        b = core // 4
        hg = core % 4
        cs = slice(hg * DL, (hg + 1) * DL)
        wq = w_qkv[:, 0 * C:1 * C][:, cs].astype(np.float16)
        wk = w_qkv[:, 1 * C:2 * C][:, cs].astype(np.float16)
        wv = w_qkv[:, 2 * C:3 * C][:, cs].astype(np.float16)
        bq = b_qkv[0 * C:1 * C][cs].astype(np.float32)
        xT = np.ascontiguousarray(x[b].T).astype(np.float16)   # [C, T]
        wp2 = w_proj[cs, :].astype(np.float16)                 # [256, 1024]
        wp_pack = np.ascontiguousarray(
            wp2.reshape(2, 128, C).transpose(1, 0, 2).reshape(128, 2 * C))
        in_maps.append({
            "x_pack": _pack8(xT),
            "wk_pack": _pack8(wk),
            "wq_pack": _pack8(wq),
            "wv_pack": _pack8(wv),
            "wp_pack": wp_pack,
            "b_q": np.stack([bq[0:128], bq[128:256]], axis=1),
            "iden16": iden,
        })
    return in_maps


def kernel(x, w_qkv, b_qkv, w_proj, b_proj, **runner_kwargs):
    x = np.asarray(x, dtype=np.float32)
    w_qkv = np.asarray(w_qkv, dtype=np.float32)
    b_qkv = np.asarray(b_qkv, dtype=np.float32)
    w_proj = np.asarray(w_proj, dtype=np.float32)
    b_proj = np.asarray(b_proj, dtype=np.float32)

    nc = _get_nc()
    in_maps = make_in_maps(x, w_qkv, b_qkv, w_proj)
    res = run_bass_kernel_spmd(nc, in_maps, list(range(N_CORES)), **runner_kwargs)
    parts = [res.results[i]["out_partial"] for i in range(N_CORES)]
    # fold V bias through the projection; K bias is softmax-invariant
    b_eff = b_proj + b_qkv[2 * C:3 * C].astype(np.float64) @ w_proj.astype(np.float64)
    outv = np.zeros((B, T, C), dtype=np.float32)
    for b in range(B):
        for hg in range(4):
            outv[b] += parts[4 * b + hg].astype(np.float32)
        outv[b] += b_eff.astype(np.float32)[None, :]
    if runner_kwargs:
        return outv, res
    return outv


if __name__ == "__main__":
    import reference

    inputs = reference.setup_inputs()
    inputs = {k: np.asarray(v) for k, v in inputs.items()}
    got = kernel(**inputs)
    want = np.asarray(reference.reference(**inputs))
    err = np.abs(got - want).max() / np.abs(want).max()
    print("rel err:", err)


# revision 18
# speedup vs baseline: 1.3619x; 1.0215x over previous
"""Multi-head self-attention Trainium2 kernel, sharded over 8 NeuronCores.

Sharding: core = (batch, head_group): 2 batches x 4 head-groups (4 heads each).
Each core computes qkv for its batch restricted to its heads, full-sequence
attention for those heads, and a row-parallel slice of the output projection,
producing a partial [T, C] output (fp16). Host: out[b] = sum of the 4
head-group partials + b_eff where b_eff folds b_proj and the V bias.

v2 design notes (all relative to the fp32/on-chip-transpose baseline):
  - x is transposed, packed and cast to fp16 on the host; no on-chip
    transposes or x^T copies are needed.
  - K bias is dropped entirely (softmax is invariant to per-query constants,
    and q.bk is per-query); V bias is folded into b_proj on the host
    (sum_s w_s = 1); only the Q bias is applied on-chip.
  - AV is computed transposed: out[q, d] = sum_s P[s,q] V[s,d] with
    ap_size=65 per chunk matmul, which halves the PE cost of AV and makes
    the softmax divide a single per-partition tensor_scalar divide.
  - The softmax denominator comes from a ones-column appended per head in
    the V tile (memset once).
  - Everything on the PE runs fp16 (1.0 cycles/row); fp8 was measured to
    break the 2e-2 gate (diffuse attention preserves per-key noise).
"""

import math
import sys

import numpy as np

sys.path.insert(0, "/opt/trn_rl_repo")

import concourse.bacc as bacc
import concourse.bass as bass
import concourse.tile as tile
from concourse import mybir
from concourse.bass_utils import run_bass_kernel_spmd

B, T, C = 2, 2048, 1024
NH, DH = 16, 64
HG = 4                  # heads per core
DL = HG * DH            # 256 local head dims
N_CORES = 8

F32 = mybir.dt.float32
F16 = mybir.dt.float16

SCALE = 1.0 / math.sqrt(DH)
Exp = mybir.ActivationFunctionType.Exp


def build_bass():
    nc = bacc.Bacc("TRN2", target_bir_lowering=False, debug=False)

    # host-packed params: [p, ci*w + j] = w[ci*128 + p, j]
    x_in = nc.declare_dram_parameter("x_pack", [128, 8 * T], F16, isOutput=False)
    wk_in = nc.declare_dram_parameter("wk_pack", [128, 8 * DL], F16, isOutput=False)
    wq_in = nc.declare_dram_parameter("wq_pack", [128, 8 * DL], F16, isOutput=False)
    wv_in = nc.declare_dram_parameter("wv_pack", [128, 8 * DL], F16, isOutput=False)
    wp_in = nc.declare_dram_parameter("wp_pack", [128, 2 * C], F16, isOutput=False)
    bq_in = nc.declare_dram_parameter("b_q", [128, 2], F32, isOutput=False)
    id_in = nc.declare_dram_parameter("iden16", [128, 128], F16, isOutput=False)
    out = nc.declare_dram_parameter("out_partial", [T, C], F16, isOutput=True)

    with tile.TileContext(nc) as tc:
        with (
            tc.tile_pool(name="singles", bufs=1) as singles,
            tc.tile_pool(name="pt", bufs=44) as ptp,
            tc.tile_pool(name="osb", bufs=6) as osbp,
            tc.tile_pool(name="oout", bufs=3) as ooutp,
            tc.tile_pool(name="sc", bufs=2, space="PSUM") as pssc,     # 2x2 banks
            tc.tile_pool(name="avp", bufs=1, space="PSUM") as psav,    # 1 bank
            tc.tile_pool(name="mm", bufs=3, space="PSUM") as psmm,     # 3x1 bank
        ):
            # ---- persistent sbuf tiles ---------------------------------
            warm = singles.tile([128, 512], F16, name="warm")
            nc.vector.memset(warm[:], 0.0)
            # pre-load the Exp activation table while DMAs are in flight
            warm_exp = singles.tile([128, 1], F16, name="warm_exp")
            nc.scalar.activation(warm_exp[:], warm[:, 0:1], Exp, scale=SCALE)

            xt = singles.tile([128, 8 * T], F16, name="xt")
            xt3 = xt[:].rearrange("p (ci t) -> p ci t", ci=8)
            xsrc = x_in[:].rearrange("p (ci t) -> p ci t", ci=8)
            NSL = 8
            TSL = T // NSL

            def x_slice(s):
                nc.sync.dma_start(
                    out=xt3[:, :, s * TSL:(s + 1) * TSL],
                    in_=xsrc[:, :, s * TSL:(s + 1) * TSL],
                )

            # DMA order tuned so the first score tile unblocks earliest:
            # Q projection (wq + x s0,s1) is the long pole for score p0.
            wq = singles.tile([128, 8 * DL], F16, name="wq")
            nc.sync.dma_start(out=wq[:], in_=wq_in[:])
            bq = singles.tile([128, 2], F32, name="bq")
            nc.sync.dma_start(out=bq[:], in_=bq_in[:])
            x_slice(0)
            x_slice(1)
            wk = singles.tile([128, 8 * DL], F16, name="wk")
            nc.sync.dma_start(out=wk[:], in_=wk_in[:])
            x_slice(2)
            x_slice(3)
            wv = singles.tile([128, 8 * DL], F16, name="wv")
            nc.sync.dma_start(out=wv[:], in_=wv_in[:])
            x_slice(4)
            x_slice(5)
            x_slice(6)
            x_slice(7)
            wp = singles.tile([128, 2 * C], F16, name="wp")
            nc.sync.dma_start(out=wp[:], in_=wp_in[:])
            iden = singles.tile([128, 128], F16, name="iden")
            nc.sync.dma_start(out=iden[:], in_=id_in[:])

            qt = [singles.tile([128, T], F16, name=f"qt{m}") for m in range(2)]
            kt = [singles.tile([128, T], F16, name=f"kt{m}") for m in range(2)]
            v_sb = [singles.tile([128, HG * (DH + 1)], F16, name=f"v{tt}")
                    for tt in range(16)]
            for tt in range(16):
                nc.vector.memset(v_sb[tt][:, DH:HG * (DH + 1):DH + 1], 1.0)
            ot = [singles.tile([128, T], F16, name=f"ot{hp}") for hp in range(2)]

            # ---- PE warmup: chew through the pstate ramp while DMAs land
            for i in range(8):
                wps = psmm.tile([128, 512], F32, tag="mm", name=f"warm{i}")
                nc.tensor.matmul(wps[:], lhsT=warm[:, 0:128], rhs=warm[:],
                                 start=True, stop=True)

            # ---- building blocks ---------------------------------------
            def k_block(km, tb):
                """K projection for 512 tokens -> kt[km][:, tb*512:...]

                Two half tiles so the psum->sbuf copy of the first 256
                tokens overlaps the second half's matmuls."""
                for half in range(2):
                    s = 2 * tb + half
                    ps = psmm.tile([128, 256], F32, tag="mm", name=f"k{km}_{s}")
                    for ci in range(8):
                        nc.tensor.matmul(
                            ps[:],
                            lhsT=wk[:, ci * 256 + km * 128: ci * 256 + (km + 1) * 128],
                            rhs=xt3[:, ci, s * 256:(s + 1) * 256],
                            start=(ci == 0),
                            stop=(ci == 7),
                        )
                    nc.vector.tensor_copy(kt[km][:, s * 256:(s + 1) * 256], ps[:])

            def q_block(qm, tb):
                for half in range(2):
                    s = 2 * tb + half
                    ps = psmm.tile([128, 256], F32, tag="mm", name=f"q{qm}_{s}")
                    for ci in range(8):
                        nc.tensor.matmul(
                            ps[:],
                            lhsT=wq[:, ci * 256 + qm * 128: ci * 256 + (qm + 1) * 128],
                            rhs=xt3[:, ci, s * 256:(s + 1) * 256],
                            start=(ci == 0),
                            stop=(ci == 7),
                        )
                    nc.vector.tensor_scalar_add(
                        qt[qm][:, s * 256:(s + 1) * 256], ps[:], bq[:, qm:qm + 1])

            def v_block(tt):
                """V projection for 128 tokens -> v_sb[tt] (65-col head blocks)"""
                ps = psmm.tile([128, 256], F32, tag="mm", name=f"v{tt}")
                for ci in range(8):
                    nc.tensor.matmul(
                        ps[:],
                        lhsT=xt3[:, ci, tt * 128:(tt + 1) * 128],
                        rhs=wv[:, ci * 256:(ci + 1) * 256],
                        start=(ci == 0),
                        stop=(ci == 7),
                    )
                dst = v_sb[tt][:].rearrange("p (h c) -> p h c", h=HG)[:, :, 0:DH]
                src = ps[:].rearrange("p (h c) -> p h c", h=HG)
                nc.vector.tensor_copy(dst, src)

            # scores tile p of unit (h, qb): key chunks 2p,2p+1 x 512 queries
            pt_tiles = {}

            def sc_tile(h, qb, p):
                km = h // 2
                row = (h % 2) * 64
                ps = pssc.tile([128, 1024], F32, tag="sc", name=f"s{h}_{qb}_{p}")
                for half in range(2):
                    st = 2 * p + half
                    nc.tensor.matmul(
                        ps[:, half * 512:(half + 1) * 512],
                        lhsT=kt[km][row:row + 64, st * 128:(st + 1) * 128],
                        rhs=qt[km][row:row + 64, qb * 512:(qb + 1) * 512],
                        start=True,
                        stop=True,
                    )
                pt = ptp.tile([128, 1024], F16, tag="pt", name=f"p{h}_{qb}_{p}")
                nc.scalar.activation(pt[:], ps[:], Exp, scale=SCALE)
                pt_tiles[(h, qb, p)] = pt

            osb_tiles = {}
            # one PSUM bank holds 4 rotating 65-col AV slots
            av_all = psav.tile([128, 4 * (DH + 1)], F32, name="av_all")
            av_ctr = [0]

            def av_group(h, qb, g):
                """AV^T for queries qtile=qb*4+g of head h -> divide into osb."""
                hp, col = h // 2, (h % 2) * 64
                slot = av_ctr[0] % 4
                av_ctr[0] += 1
                av = av_all[:, slot * (DH + 1):(slot + 1) * (DH + 1)]
                for st in range(16):
                    ptk = pt_tiles[(h, qb, st // 2)]
                    nc.tensor.matmul(
                        av[:],
                        lhsT=ptk[:, (st % 2) * 512 + g * 128:
                                 (st % 2) * 512 + (g + 1) * 128],
                        rhs=v_sb[st][:, h * (DH + 1):(h + 1) * (DH + 1)],
                        start=(st == 0),
                        stop=(st == 15),
                    )
                key = (hp, qb, g)
                if key not in osb_tiles:
                    osb_tiles[key] = osbp.tile([128, 128], F16, tag="osb",
                                               name=f"o{hp}_{qb}_{g}")
                rec = osbp.tile([128, 1], F32, tag="rec", bufs=4,
                                name=f"r{h}_{qb}_{g}")
                nc.vector.reciprocal(rec[:], av[:, DH:DH + 1])
                nc.vector.tensor_scalar_mul(
                    osb_tiles[key][:, col:col + 64], av[:, 0:DH], rec[:, 0:1])

            def transpose_hp(hp, qb):
                """osb pair tiles (4 qtiles) -> ot[hp][:, qb*512:...]"""
                ps = psmm.tile([128, 512], F16, tag="mm", name=f"t{hp}_{qb}")
                for g in range(4):
                    nc.tensor.transpose(
                        ps[:, g * 128:(g + 1) * 128],
                        osb_tiles[(hp, qb, g)][:],
                        iden[:],
                    )
                nc.vector.tensor_copy(ot[hp][:, qb * 512:(qb + 1) * 512], ps[:])

            Copy = mybir.ActivationFunctionType.Copy

            def proj_tile(tt, use_act=False):
                o_out = ooutp.tile([128, C], F16, tag="oout", name=f"oo{tt}")
                for nb in range(2):
                    ps = psmm.tile([128, 512], F32, tag="mm", name=f"pr{tt}_{nb}")
                    for hp in range(2):
                        nc.tensor.matmul(
                            ps[:],
                            lhsT=ot[hp][:, tt * 128:(tt + 1) * 128],
                            rhs=wp[:, hp * C + nb * 512: hp * C + (nb + 1) * 512],
                            start=(hp == 0),
                            stop=(hp == 1),
                        )
                    dst = o_out[:, nb * 512:(nb + 1) * 512]
                    if use_act:
                        # tail: ACT is idle after the last exp, DVE is not;
                        # half-DMAs overlap the copy of the other half
                        nc.scalar.activation(dst, ps[:], Copy)
                        nc.sync.dma_start(
                            out=out[tt * 128:(tt + 1) * 128,
                                    nb * 512:(nb + 1) * 512],
                            in_=dst)
                    else:
                        nc.vector.tensor_copy(dst, ps[:])
                if not use_act:
                    nc.sync.dma_start(out=out[tt * 128:(tt + 1) * 128, :],
                                      in_=o_out[:])

            # ---- fill queue: transposes + proj consumed in spare PE slots
            fillq = []
            done_av = set()

            def maybe_posts(av_u):
                qb, h = av_u // 4, av_u % 4
                if h == 1:
                    fillq.append(lambda qb=qb: transpose_hp(0, qb))
                elif h == 3 and qb < 3:
                    fillq.append(lambda qb=qb: transpose_hp(1, qb))
                    for tt in range(4 * qb, 4 * qb + 4):
                        fillq.append(lambda tt=tt: proj_tile(tt))

            def full_av(av_u):
                for g in range(4):
                    av_group(av_u % 4, av_u // 4, g)
                done_av.add(av_u)
                maybe_posts(av_u)

            # ---- intro: K + qb0 scores dominate; only 12 V blocks and the
            # qb0 Q blocks live here so ACT stays fed from the start.
            # heads 0,1 share K/Q m-block 0, so their 4 score tiles can all
            # fire right after K0 (+Q0); K1/Q1/V hide under those exps.
            IV = {0: [0], 1: [1, 2, 3], 2: [4, 5, 6], 3: [7, 8, 9, 10, 11]}
            for tb in range(4):
                vq = list(IV[tb])
                if tb == 0:
                    q_block(0, 0)
                k_block(0, tb)
                sc_tile(0, 0, 2 * tb)
                sc_tile(0, 0, 2 * tb + 1)
                sc_tile(1, 0, 2 * tb)
                sc_tile(1, 0, 2 * tb + 1)
                k_block(1, tb)
                if tb == 0:
                    q_block(1, 0)
                if vq:
                    v_block(vq.pop(0))
                sc_tile(2, 0, 2 * tb)
                if vq:
                    v_block(vq.pop(0))
                sc_tile(2, 0, 2 * tb + 1)
                sc_tile(3, 0, 2 * tb)
                if vq:
                    v_block(vq.pop(0))
                sc_tile(3, 0, 2 * tb + 1)
                while vq:
                    v_block(vq.pop(0))

            # ---- steady state: units u = qb*4 + h -----------------------
            pre_fills = {4: [lambda: q_block(0, 1)]}
            mid_fills = {5: [lambda: q_block(1, 1)],
                         6: [lambda: q_block(0, 2)],
                         7: [lambda: q_block(1, 2)],
                         8: [lambda: q_block(0, 3)],
                         9: [lambda: q_block(1, 3)]}
            unit_v = {4: [12, 13, 14, 15]}
            av_plan = {5: [0], 6: [1], 7: [2, 3], 8: [4, 5], 9: [6, 7],
                       10: [8], 11: [9], 12: [10], 13: [11], 14: [12],
                       15: [13, 14]}

            def emit_unit(u):
                qb, h = u // 4, u % 4
                for f in pre_fills.get(u, []):
                    f()
                avs = av_plan.get(u, [])
                first = avs[0] if avs else None
                extras = [lambda tt=tt: v_block(tt) for tt in unit_v.get(u, [])]
                extras += mid_fills.get(u, [])
                nfill = 2
                for p in range(8):
                    sc_tile(h, qb, p)
                    if first is not None and 2 <= p <= 5:
                        g = p - 2
                        av_group(first % 4, first // 4, g)
                        if g == 3:
                            done_av.add(first)
                            maybe_posts(first)
                    elif extras:
                        extras.pop(0)()
                    elif nfill and fillq:
                        nfill -= 1
                        fillq.pop(0)()
                while extras:
                    extras.pop(0)()
                for av_u in avs[1:]:
                    full_av(av_u)

            for u in range(4, 16):
                emit_unit(u)
            # drain: flush pending fills (incl. tp(0,3)), then pipeline the
            # last unit per qtile: AV group -> transpose column -> proj tile.
            while fillq:
                fillq.pop(0)()
            for g in range(4):
                av_group(3, 3, g)
                tps = psmm.tile([128, 128], F16, tag="mm", name=f"tpg{g}")
                nc.tensor.transpose(tps[:], osb_tiles[(1, 3, g)][:], iden[:])
                nc.vector.tensor_copy(
                    ot[1][:, 1536 + g * 128:1536 + (g + 1) * 128], tps[:])
                proj_tile(12 + g, use_act=True)

    nc.compile()
    return nc


_CACHE = {}


def _get_nc():
    if "nc" not in _CACHE:
        _CACHE["nc"] = build_bass()
    return _CACHE["nc"]


def _pack8(w):
    """[1024, n] -> [128, 8*n] with [p, ci*n+j] = w[ci*128+p, j]"""
    n = w.shape[1]
    return np.ascontiguousarray(
        w.reshape(8, 128, n).transpose(1, 0, 2).reshape(128, 8 * n))


def make_in_maps(x, w_qkv, b_qkv, w_proj):
    iden = np.eye(128, dtype=np.float16)
    in_maps = []
    for core in range(N_CORES):
        b = core // 4
        hg = core % 4
        cs = slice(hg * DL, (hg + 1) * DL)
        wq = w_qkv[:, 0 * C:1 * C][:, cs].astype(np.float16)
        wk = w_qkv[:, 1 * C:2 * C][:, cs].astype(np.float16)
        wv = w_qkv[:, 2 * C:3 * C][:, cs].astype(np.float16)
        bq = b_qkv[0 * C:1 * C][cs].astype(np.float32)
        xT = np.ascontiguousarray(x[b].T).astype(np.float16)   # [C, T]
        wp2 = w_proj[cs, :].astype(np.float16)                 # [256, 1024]
        wp_pack = np.ascontiguousarray(
            wp2.reshape(2, 128, C).transpose(1, 0, 2).reshape(128, 2 * C))
        in_maps.append({
            "x_pack": _pack8(xT),
            "wk_pack": _pack8(wk),
            "wq_pack": _pack8(wq),
            "wv_pack": _pack8(wv),
            "wp_pack": wp_pack,
            "b_q": np.stack([bq[0:128], bq[128:256]], axis=1),
            "iden16": iden,
        })
    return in_maps


def kernel(x, w_qkv, b_qkv, w_proj, b_proj, **runner_kwargs):
    x = np.asarray(x, dtype=np.float32)
    w_qkv = np.asarray(w_qkv, dtype=np.float32)
    b_qkv = np.asarray(b_qkv, dtype=np.float32)
    w_proj = np.asarray(w_proj, dtype=np.float32)
    b_proj = np.asarray(b_proj, dtype=np.float32)

    nc = _get_nc()
    in_maps = make_in_maps(x, w_qkv, b_qkv, w_proj)
    res = run_bass_kernel_spmd(nc, in_maps, list(range(N_CORES)), **runner_kwargs)
    parts = [res.results[i]["out_partial"] for i in range(N_CORES)]
    # fold V bias through the projection; K bias is softmax-invariant
    b_eff = b_proj + b_qkv[2 * C:3 * C].astype(np.float64) @ w_proj.astype(np.float64)
    outv = np.zeros((B, T, C), dtype=np.float32)
    for b in range(B):
        for hg in range(4):
            outv[b] += parts[4 * b + hg].astype(np.float32)
        outv[b] += b_eff.astype(np.float32)[None, :]
    if runner_kwargs:
        return outv, res
    return outv


if __name__ == "__main__":
    import reference

    inputs = reference.setup_inputs()
    inputs = {k: np.asarray(v) for k, v in inputs.items()}
    got = kernel(**inputs)
    want = np.asarray(reference.reference(**inputs))
    err = np.abs(got - want).max() / np.abs(want).max()
    print("rel err:", err)
